# revision 26
# baseline (speedup 1.0000x reference)
"""Per-pixel dynamic 7x7 filtering (BaseTextureDiffusion._diffusion_step)
on 8 Trainium2 NeuronCores.

out[b,c,h,w] = sum_k weights[b,c,k,h,w] * pad_edge(latent)[b,c,h+i,w+j],
k = i*7+j.

Sharding: the 48 (b,c) planes are independent -> 6 planes per core.
Latent is replicate-padded on host (tiny) so the device kernel does no
edge handling.

Device layout per core: partition dim = image rows (2 blocks of 128),
free dim = (plane, col).  Inputs are shipped as fp16 (half the HBM
traffic of f32; rel err ~6e-4 vs the 2e-2 budget).

Engine split (the key change vs the pure-DVE baseline): the vector
engine computes ONLY the 49 per-tap products prod_k = w_k * x_shift_k
(fp16, 2x mode).  The 48 accumulation adds are moved to the otherwise
idle TensorEngine as identity matmuls with PSUM accumulation
(start/stop flags): psum += I.T @ prod_k.  PSUM accumulates in f32, so
accuracy is slightly better than the baseline's fp16 group partials.

Column shifts: DVE 2x mode requires 4B-aligned reads.  Odd col shifts j
are handled by pre-shifting those weight planes one column on the host
(w stored at col+1) so the latent read offset becomes j-1 (even); the
product then lands one column to the right and the matmul rhs slice
[1:257) realigns it into psum.  This removes the baseline's second
one-column-shifted copy of the latent (halves latent HBM traffic).

Row shifts i stay as 7 row-shifted latent tile loads per 128-row block
(a partition shift needs a DMA; deriving them SBUF->SBUF measured worse
- the fabric ports bind before HBM does).

Measured bottleneck: DMA (~45.3 MB/core/invocation; dma_only == full
kernel time), sustaining ~370 GB/s/core ~= the device HBM roofline,
with DVE (~80 us) and PE (~63 us) fully hidden underneath.  Weight
DMAs are one contiguous 2.8 MB transfer per (128-row block, tap row);
latent/output use [row, plane, col] layouts so every partition line is
one contiguous run.  DVE muls are packed 4-or-3-taps-per-instruction
via overlapping stride-2 window APs (host stores taps parity-ordered).
Sub-fp16 weight formats were evaluated and rejected: SWDGE cast-DMA
(u8->f16) fails walrus codegen in this container, and every engine
widening path is slower than the DVE mul itself.
"""

import numpy as np

B, C, H, W = 2, 24, 256, 256
R = 7  # window size
PAD = R // 2  # 3
NCORES = 8
PLANES = B * C  # 48
PPC = PLANES // NCORES  # 6 planes per core
HX = H + 2 * PAD  # 262 padded rows
WX = H + PAD + 5  # 264 padded cols (3 left, 5 right; even for alignment)
WV = W + 2  # 258 weight cols (1 shift col + 1 pad col; even)
NSEG = 3  # psum segments of 512 f32 (one bank each) covering PPC*W=1536
# Weight HBM format: 'f16', or 'u8' = uint8-quantized (q=round(w*255)),
# widened to fp16 by a casting SWDGE DMA; the 1/255 dequant scale is folded
# into the host-side latent.  Halves HBM-side weight bytes; quantization
# adds ~2e-3 rel err (budget 2e-2).
WDTYPE = "f16"
# Packed muls: host stores each tap-row's 7 taps in parity order
# [0,2,4,6,1,3,5]; the device then computes all 4 even-parity (resp. 3
# odd-parity) products of a tap-row in ONE DVE op, reading the latent
# through an overlapping stride-2 window AP (saves per-op overhead and
# cuts DVE instruction count 98 -> 28).
PACKED = True
TAP_ORDER = (0, 2, 4, 6, 1, 3, 5)

_cache = {}


def _split_multi_waits(nc, max_waits: int = 1):
    """walrus CoreV3 codegen in this container rejects instructions carrying
    more than one sync wait ('Too many sync wait commands').  Legalize the
    module by hoisting extra waits onto same-engine NoOps inserted directly
    before the instruction (engine stalls at the nop first — semantics
    preserved, the instruction still executes only after all conditions)."""
    import concourse.mybir as mybir

    cnt = 0
    for f in nc.m.functions:
        for b in f.blocks:
            changed = False
            new_insts = []
            for inst in b.instructions:
                si = inst.sync_info
                if si is not None and len(si.on_wait) > max_waits:
                    waits = list(si.on_wait)
                    upds = list(si.on_update)
                    chunks = [
                        waits[i : i + max_waits]
                        for i in range(0, len(waits), max_waits)
                    ]
                    for chunk in chunks[:-1]:
                        nop = mybir.InstNoOp(
                            name=f"ws_nop_{cnt}", ins=[], outs=[]
                        )
                        cnt += 1
                        nop.engine = inst.engine
                        nop.sync_info = mybir.SyncInfo(
                            on_wait=chunk, on_update=[]
                        )
                        new_insts.append(nop)
                    inst.sync_info = mybir.SyncInfo(
                        on_wait=chunks[-1], on_update=upds
                    )
                    changed = True
                new_insts.append(inst)
            if changed:
                b.instructions = new_insts


def build_nc(
    reps: int = 1,
    loop_reps: int | None = None,
    legalize: bool = True,
    row_group: bool = True,
    dma_only: bool = False,
    staggered: bool = True,
    wdtype: str = WDTYPE,
    packed: bool = PACKED,
    bufs_wg: int = 3,
    bufs_prod: int = 3,
):
    """Build the per-core Bass program (SPMD; all cores run the same NEFF).

    loop_reps: if set, wrap ONE rep body in a hardware For_i loop with this
    trip count (constant NEFF size for any count; used for timing).
    """
    import concourse.bass as bass
    import concourse.mybir as mybir
    from concourse import masks
    from concourse.tile import TileContext

    dt = mybir.dt.float16
    dto = mybir.dt.float32
    dtw = mybir.dt.uint8 if wdtype == "u8" else dt

    nc = bass.Bass("TRN2", target_bir_lowering=False, debug=False, num_devices=NCORES)
    # Weights pre-transposed on host to [row, k, plane, col] so each
    # (row-block, tap) DMA is contiguous per partition; odd-j taps are
    # pre-shifted one col right (see module docstring).
    wt = nc.dram_tensor("wt", [H, R * R, PPC, WV], dtw, kind="ExternalInput").ap()
    # latent/output stored [row, plane, col] on host so every x-tile DMA
    # reads (and the out DMA writes) one contiguous run per partition
    lp_r = nc.dram_tensor("lp", [HX, PPC, WX], dt, kind="ExternalInput").ap()
    out_r = nc.dram_tensor("out", [H, PPC, W], dt, kind="ExternalOutput").ap()

    with TileContext(nc) as tc:
        with (
            tc.tile_pool(name="pool", bufs=1) as pool,
            tc.tile_pool(name="psum", bufs=1, space="PSUM") as psum_pool,
        ):
            # 128x128 fp16 identity: stationary operand for the accumulate
            # matmuls.  Built once on gpsimd (idle otherwise), outside the
            # timing loop.
            ident = pool.tile([128, 128], dt, name="ident", tag="ident", bufs=1)
            masks.make_identity(nc, ident[:])

            def rep_body(rep):
                for blk in range(H // 128):
                    r0 = blk * 128
                    # Row-shifted padded-latent tiles, loaded lazily right
                    # before the first tap that needs them.
                    rs = {}

                    def need_row(i):
                        if i in rs:
                            return
                        t = pool.tile(
                            [128, PPC, WX], dt,
                            name=f"x_{rep}_{blk}_{i}", tag=f"x{i}", bufs=2,
                        )
                        # x/out DMAs issue on the ACT HWDGE ring; weights on
                        # SP — two independent descriptor-generation queues
                        nc.scalar.dma_start(out=t[:], in_=lp_r[r0 + i : r0 + i + 128])
                        rs[i] = t

                    psums = (
                        None
                        if dma_only
                        else [
                            psum_pool.tile(
                                [128, 512], dto,
                                name=f"ps{s}_{rep}_{blk}", tag=f"ps{s}", bufs=2,
                            )
                            for s in range(NSEG)
                        ]
                    )
                    # casting (u8->f16) DMAs must go through SWDGE (gpsimd)
                    weng = nc.gpsimd if wdtype == "u8" else nc.sync
                    if packed:
                        from concourse.ap import AP

                        for i in range(R):
                            need_row(i)
                            wg = pool.tile(
                                [128, R, PPC, WV], dt,
                                name=f"w_{rep}_{blk}_{i}", tag="wg", bufs=bufs_wg,
                            )
                            weng.dma_start(
                                out=wg[:], in_=wt[r0 : r0 + 128, R * i : R * i + R]
                            )
                            if dma_only:
                                continue
                            xb = rs[i][:]
                            # (start member, count, psum col offset) per parity
                            for gi, (t0, cnt, off) in enumerate(
                                ((0, 4, 0), (4, 3, 1))
                            ):
                                prod = pool.tile(
                                    [128, cnt, PPC, WV], dt,
                                    name=f"p_{rep}_{blk}_{i}_{gi}",
                                    tag=f"prod{gi}", bufs=bufs_prod,
                                )
                                # overlapping stride-2 window view of the
                                # latent tile: member t reads cols [2t, 2t+WV)
                                xap = AP(
                                    xb.tensor, xb.offset,
                                    [[PPC * WX, 128], [2, cnt], [WX, PPC], [1, WV]],
                                )
                                nc.vector.tensor_mul(
                                    prod[:], wg[:, t0 : t0 + cnt], xap
                                )
                                for m in range(cnt):
                                    for s in range(NSEG):
                                        nc.tensor.matmul(
                                            psums[s][:],
                                            ident[:],
                                            prod[:, m, 2 * s : 2 * s + 2, off : off + W],
                                            start=(i == 0 and gi == 0 and m == 0),
                                            stop=(
                                                i == R - 1 and gi == 1 and m == cnt - 1
                                            ),
                                        )
                    wgs = {}  # i -> row-group weight tile (row_group mode)
                    for k in range(R * R if not packed else 0):
                        i, j = divmod(k, R)
                        need_row(i)
                        if row_group:
                            # one 2.8 MB DMA per tap-row: the 7 taps of row i
                            # are contiguous in the host weight layout
                            if i not in wgs:
                                wg = pool.tile(
                                    [128, R, PPC, WV], dt,
                                    name=f"w_{rep}_{blk}_{i}", tag="wg", bufs=bufs_wg,
                                )
                                weng.dma_start(
                                    out=wg[:], in_=wt[r0 : r0 + 128, R * i : R * i + R]
                                )
                                wgs[i] = wg
                            wk_ap = wgs[i][:, j]
                        else:
                            wk = pool.tile(
                                [128, PPC, WV], dt,
                                name=f"w_{rep}_{blk}_{k}", tag="wk", bufs=6,
                            )
                            weng.dma_start(out=wk[:], in_=wt[r0 : r0 + 128, k])
                            wk_ap = wk[:]
                        if dma_only:
                            continue
                        prod = pool.tile(
                            [128, PPC, WV], dt,
                            name=f"p_{rep}_{blk}_{k}", tag="prod", bufs=4,
                        )
                        c0 = j - (j % 2)  # even latent col offset
                        nc.vector.tensor_mul(
                            prod[:], wk_ap, rs[i][:, :, c0 : c0 + WV]
                        )
                        off = j % 2  # product cols [off, off+W) hold out cols [0, W)
                        for s in range(NSEG):
                            nc.tensor.matmul(
                                psums[s][:],
                                ident[:],
                                prod[:, 2 * s : 2 * s + 2, off : off + W],
                                start=(k == 0),
                                stop=(k == R * R - 1),
                            )
                    outsb = pool.tile(
                        [128, PPC, W], dt,
                        name=f"o_{rep}_{blk}", tag="outsb", bufs=2,
                    )
                    if dma_only:
                        nc.gpsimd.memset(outsb[:], 0.0)
                    else:
                        for s in range(NSEG):
                            nc.scalar.copy(
                                outsb[:, 2 * s : 2 * s + 2, :],
                                psums[s][:].rearrange("p (a b) -> p a b", a=2),
                            )
                    nc.scalar.dma_start(out=out_r[r0 : r0 + 128], in_=outsb[:])

            if loop_reps is not None:
                with tc.For_i(0, loop_reps, 1, staggered_reset=staggered):
                    rep_body(0)
            else:
                for rep in range(reps):
                    rep_body(rep)
    if legalize:
        _split_multi_waits(nc)
    return nc


def _prep_inputs(latent, weights, wdtype: str = WDTYPE, packed: bool = PACKED):
    lat = np.asarray(latent, dtype=np.float32).reshape(PLANES, H, W)
    wts = np.asarray(weights, dtype=np.float32).reshape(PLANES, R * R, H, W)
    if wdtype == "u8":
        # w ~= q/255 with q uint8; the 1/255 is folded into the latent below
        lat = lat * np.float32(1.0 / 255.0)
    # Edge-pad latent: rows 3/3, cols 3 left / 5 right (extra 2 right cols
    # keep the 258-wide device reads in-bounds; values there only feed
    # zero-weight product columns).
    lpad = np.pad(lat, ((0, 0), (PAD, PAD), (PAD, 5)), mode="edge").astype(np.float16)
    odd = (np.arange(R * R) % R) % 2 == 1
    in_maps = []
    for c in range(NCORES):
        wc = wts[c * PPC : (c + 1) * PPC]  # [6, 49, 256, 256]
        wt_t = wc.transpose(2, 1, 0, 3)  # [row, k, plane, col]
        if wdtype == "u8":
            wt_t = np.clip(np.round(wt_t * 255.0), 0, 255)
            v = np.zeros((H, R * R, PPC, WV), dtype=np.uint8)
        else:
            v = np.zeros((H, R * R, PPC, WV), dtype=np.float16)
        v[:, ~odd, :, 0:W] = wt_t[:, ~odd]
        v[:, odd, :, 1 : W + 1] = wt_t[:, odd]  # odd col taps pre-shifted
        if packed:
            perm = [R * i + j for i in range(R) for j in TAP_ORDER]
            v = v[:, perm]
        in_maps.append(
            {
                "wt": np.ascontiguousarray(v),
                "lp": np.ascontiguousarray(
                    lpad[c * PPC : (c + 1) * PPC].transpose(1, 0, 2)
                ),
            }
        )
    return in_maps


def _get_runner():
    """Build the Bass program and ONE sharded jit executable, cached for the
    process.  Repeated kernel() calls reuse the same loaded executable —
    creating a fresh jit per call (as run_bass_kernel_spmd does) loads a new
    executable each time and can wedge the device on the second call."""
    if "runner" in _cache:
        return _cache["runner"]

    import jax
    import concourse.mybir as mybir
    from concourse import bass2jax
    from jax.experimental.shard_map import shard_map
    from jax.sharding import Mesh, NamedSharding, PartitionSpec

    bass2jax.install_neuronx_cc_hook()
    nc = build_nc(reps=1)

    partition_name = nc.partition_id_tensor.name if nc.partition_id_tensor else None
    in_names, out_names, out_avals, zero_outs = [], [], [], []
    for alloc in nc.m.functions[0].allocations:
        if not isinstance(alloc, mybir.MemoryLocationSet):
            continue
        name = alloc.memorylocations[0].name
        if alloc.kind == "ExternalInput":
            if name != partition_name:
                in_names.append(name)
        elif alloc.kind == "ExternalOutput":
            out_names.append(name)
            shape = tuple(alloc.tensor_shape)
            dtype = mybir.dt.np(alloc.dtype)
            out_avals.append(jax.core.ShapedArray(shape, dtype))
            zero_outs.append(np.zeros(shape, dtype))
    n_params = len(in_names)
    all_in_names = list(in_names) + out_names
    if partition_name is not None:
        all_in_names.append(partition_name)

    def _body(*args):
        operands = list(args)
        if partition_name is not None:
            operands.append(bass2jax.partition_id_tensor())
        return tuple(
            bass2jax._bass_exec_p.bind(
                *operands,
                out_avals=tuple(out_avals),
                in_names=tuple(all_in_names),
                out_names=tuple(out_names),
                lowering_input_output_aliases=(),
                sim_require_finite=True,
                sim_require_nnan=True,
                nc=nc,
            )
        )

    devices = jax.devices()[:NCORES]
    mesh = Mesh(np.asarray(devices), ("core",))
    in_specs = (PartitionSpec("core"),) * (n_params + len(out_names))
    out_specs = (PartitionSpec("core"),) * len(out_names)
    sharded = jax.jit(
        shard_map(
            _body, mesh=mesh, in_specs=in_specs, out_specs=out_specs, check_rep=False
        ),
        keep_unused=True,
    )
    sh = NamedSharding(mesh, PartitionSpec("core"))
    zeros_dev = [
        jax.device_put(np.zeros((NCORES * z.shape[0], *z.shape[1:]), z.dtype), sh)
        for z in zero_outs
    ]

    def run(in_maps):
        ins_dev = [
            jax.device_put(
                np.concatenate([in_maps[c][n] for c in range(NCORES)], axis=0), sh
            )
            for n in in_names
        ]
        outs = sharded(*ins_dev, *zeros_dev)
        jax.block_until_ready(outs)
        # one output tensor: per-core [PPC, H, W] concatenated on axis 0
        return np.asarray(outs[0])

    _cache["runner"] = run
    return run


def kernel(latent, weights, window_size):
    r = int(window_size)
    assert r == R, f"kernel hardcoded for window_size={R}, got {r}"

    run = _get_runner()
    in_maps = _prep_inputs(latent, weights)
    full = run(in_maps)  # [NCORES*H, PPC, W]
    full = full.reshape(NCORES, H, PPC, W).transpose(0, 2, 1, 3)
    return full.reshape(B, C, H, W).astype(np.float32)


# revision 27
# speedup vs baseline: 1.0807x; 1.0807x over previous
"""Per-pixel dynamic 7x7 filtering (BaseTextureDiffusion._diffusion_step)
on 8 Trainium2 NeuronCores.

out[b,c,h,w] = sum_k weights[b,c,k,h,w] * pad_edge(latent)[b,c,h+i,w+j],
k = i*7+j.

Sharding: the 48 (b,c) planes are independent -> 6 planes per core.
Latent is replicate-padded on host (tiny) so the device kernel does no
edge handling.

Device layout per core: partition dim = image rows (2 blocks of 128),
free dim = (plane, col).  Inputs are shipped as fp16 (half the HBM
traffic of f32; rel err ~6e-4 vs the 2e-2 budget).

Engine split (the key change vs the pure-DVE baseline): the vector
engine computes ONLY the 49 per-tap products prod_k = w_k * x_shift_k
(fp16, 2x mode).  The 48 accumulation adds are moved to the otherwise
idle TensorEngine as identity matmuls with PSUM accumulation
(start/stop flags): psum += I.T @ prod_k.  PSUM accumulates in f32, so
accuracy is slightly better than the baseline's fp16 group partials.

Column shifts: DVE 2x mode requires 4B-aligned reads.  Odd col shifts j
are handled by pre-shifting those weight planes one column on the host
(w stored at col+1) so the latent read offset becomes j-1 (even); the
product then lands one column to the right and the matmul rhs slice
[1:257) realigns it into psum.  This removes the baseline's second
one-column-shifted copy of the latent (halves latent HBM traffic).

Row shifts i stay as 7 row-shifted latent tile loads per 128-row block
(a partition shift needs a DMA; deriving them SBUF->SBUF measured worse
- the fabric ports bind before HBM does).

Measured bottleneck: DMA (~45.3 MB/core/invocation; dma_only == full
kernel time), sustaining ~370 GB/s/core ~= the device HBM roofline,
with DVE (~80 us) and PE (~63 us) fully hidden underneath.  Weight
DMAs are one contiguous 2.8 MB transfer per (128-row block, tap row);
latent/output use [row, plane, col] layouts so every partition line is
one contiguous run.  DVE muls are packed 4-or-3-taps-per-instruction
via overlapping stride-2 window APs (host stores taps parity-ordered).
Sub-fp16 weight formats were evaluated and rejected: SWDGE cast-DMA
(u8->f16) fails walrus codegen in this container, and every engine
widening path is slower than the DVE mul itself.
"""

import numpy as np

B, C, H, W = 2, 24, 256, 256
R = 7  # window size
PAD = R // 2  # 3
NCORES = 8
PLANES = B * C  # 48
PPC = PLANES // NCORES  # 6 planes per core
HX = H + 2 * PAD  # 262 padded rows
WX = H + PAD + 5  # 264 padded cols (3 left, 5 right; even for alignment)
WV = W + 2  # 258 weight cols (1 shift col + 1 pad col; even)
NSEG = 3  # psum segments of 512 f32 (one bank each) covering PPC*W=1536
# Weight HBM format: 'f16', or 'u8' = uint8-quantized (q=round(w*255)),
# widened to fp16 by a casting SWDGE DMA; the 1/255 dequant scale is folded
# into the host-side latent.  Halves HBM-side weight bytes; quantization
# adds ~2e-3 rel err (budget 2e-2).
WDTYPE = "f16"
# Packed muls: host stores each tap-row's 7 taps in parity order
# [0,2,4,6,1,3,5]; the device then computes all 4 even-parity (resp. 3
# odd-parity) products of a tap-row in ONE DVE op, reading the latent
# through an overlapping stride-2 window AP (saves per-op overhead and
# cuts DVE instruction count 98 -> 28).
PACKED = True
TAP_ORDER = (0, 2, 4, 6, 1, 3, 5)

_cache = {}


def _split_multi_waits(nc, max_waits: int = 1):
    """walrus CoreV3 codegen in this container rejects instructions carrying
    more than one sync wait ('Too many sync wait commands').  Legalize the
    module by hoisting extra waits onto same-engine NoOps inserted directly
    before the instruction (engine stalls at the nop first — semantics
    preserved, the instruction still executes only after all conditions)."""
    import concourse.mybir as mybir

    cnt = 0
    for f in nc.m.functions:
        for b in f.blocks:
            changed = False
            new_insts = []
            for inst in b.instructions:
                si = inst.sync_info
                if si is not None and len(si.on_wait) > max_waits:
                    waits = list(si.on_wait)
                    upds = list(si.on_update)
                    chunks = [
                        waits[i : i + max_waits]
                        for i in range(0, len(waits), max_waits)
                    ]
                    for chunk in chunks[:-1]:
                        nop = mybir.InstNoOp(
                            name=f"ws_nop_{cnt}", ins=[], outs=[]
                        )
                        cnt += 1
                        nop.engine = inst.engine
                        nop.sync_info = mybir.SyncInfo(
                            on_wait=chunk, on_update=[]
                        )
                        new_insts.append(nop)
                    inst.sync_info = mybir.SyncInfo(
                        on_wait=chunks[-1], on_update=upds
                    )
                    changed = True
                new_insts.append(inst)
            if changed:
                b.instructions = new_insts


def build_nc(
    reps: int = 1,
    loop_reps: int | None = None,
    legalize: bool = True,
    row_group: bool = True,
    dma_only: bool = False,
    staggered: bool = True,
    wdtype: str = WDTYPE,
    packed: bool = PACKED,
    bufs_wg: int = 3,
    bufs_prod: int = 3,
    x_merge: bool = True,
    w_split: bool = False,
):
    """Build the per-core Bass program (SPMD; all cores run the same NEFF).

    loop_reps: if set, wrap ONE rep body in a hardware For_i loop with this
    trip count (constant NEFF size for any count; used for timing).
    """
    import concourse.bass as bass
    import concourse.mybir as mybir
    from concourse import masks
    from concourse.tile import TileContext

    dt = mybir.dt.float16
    dto = mybir.dt.float32
    dtw = mybir.dt.uint8 if wdtype == "u8" else dt

    nc = bass.Bass("TRN2", target_bir_lowering=False, debug=False, num_devices=NCORES)
    # Weights pre-transposed on host to [row, k, plane, col] so each
    # (row-block, tap) DMA is contiguous per partition; odd-j taps are
    # pre-shifted one col right (see module docstring).
    wt = nc.dram_tensor("wt", [H, R * R, PPC, WV], dtw, kind="ExternalInput").ap()
    # latent/output stored [row, plane, col] on host so every x-tile DMA
    # reads (and the out DMA writes) one contiguous run per partition
    lp_r = nc.dram_tensor("lp", [HX, PPC, WX], dt, kind="ExternalInput").ap()
    out_r = nc.dram_tensor("out", [H, PPC, W], dt, kind="ExternalOutput").ap()

    with TileContext(nc) as tc:
        with (
            tc.tile_pool(name="pool", bufs=1) as pool,
            tc.tile_pool(name="psum", bufs=1, space="PSUM") as psum_pool,
        ):
            # 128x128 fp16 identity: stationary operand for the accumulate
            # matmuls.  Built once on gpsimd (idle otherwise), outside the
            # timing loop.
            ident = pool.tile([128, 128], dt, name="ident", tag="ident", bufs=1)
            masks.make_identity(nc, ident[:])

            def rep_body(rep):
                for blk in range(H // 128):
                    r0 = blk * 128
                    # Row-shifted padded-latent tiles, loaded lazily right
                    # before the first tap that needs them.
                    rs = {}

                    def need_row(i):
                        if i in rs:
                            return
                        t = pool.tile(
                            [128, PPC, WX], dt,
                            name=f"x_{rep}_{blk}_{i}", tag=f"x{i}", bufs=2,
                        )
                        # x/out DMAs issue on the ACT HWDGE ring; weights on
                        # SP — two independent descriptor-generation queues
                        nc.scalar.dma_start(out=t[:], in_=lp_r[r0 + i : r0 + i + 128])
                        rs[i] = t

                    psums = (
                        None
                        if dma_only
                        else [
                            psum_pool.tile(
                                [128, 512], dto,
                                name=f"ps{s}_{rep}_{blk}", tag=f"ps{s}", bufs=2,
                            )
                            for s in range(NSEG)
                        ]
                    )
                    # casting (u8->f16) DMAs must go through SWDGE (gpsimd)
                    weng = nc.gpsimd if wdtype == "u8" else nc.sync
                    if packed:
                        from concourse.ap import AP

                        xall = None
                        if x_merge:
                            # ONE x DMA per block: slot (p, i) holds padded
                            # row r0+p+i — the source AP re-reads overlapping
                            # HBM rows, so the 6-row halo comes free
                            xall = pool.tile(
                                [128, R, PPC, WX], dt,
                                name=f"xa_{rep}_{blk}", tag="xa", bufs=2,
                            )
                            xbase = lp_r[r0 : r0 + 128]
                            nc.scalar.dma_start(
                                out=xall[:],
                                in_=AP(
                                    xbase.tensor, xbase.offset,
                                    [
                                        [PPC * WX, 128],
                                        [PPC * WX, R],
                                        [WX, PPC],
                                        [1, WX],
                                    ],
                                ),
                            )
                        for i in range(R):
                            if not x_merge:
                                need_row(i)
                            wg = pool.tile(
                                [128, R, PPC, WV], dt,
                                name=f"w_{rep}_{blk}_{i}", tag="wg", bufs=bufs_wg,
                            )
                            # optionally alternate weight DMAs across the two
                            # HWDGE rings (SP / ACT) to cut ring-level HOL
                            we = nc.scalar if (w_split and i % 2) else weng
                            we.dma_start(
                                out=wg[:], in_=wt[r0 : r0 + 128, R * i : R * i + R]
                            )
                            if dma_only:
                                continue
                            xb = xall[:, i] if x_merge else rs[i][:]
                            # (start member, count, psum col offset) per parity
                            for gi, (t0, cnt, off) in enumerate(
                                ((0, 4, 0), (4, 3, 1))
                            ):
                                prod = pool.tile(
                                    [128, cnt, PPC, WV], dt,
                                    name=f"p_{rep}_{blk}_{i}_{gi}",
                                    tag=f"prod{gi}", bufs=bufs_prod,
                                )
                                # overlapping stride-2 window view of the
                                # latent tile: member t reads cols [2t, 2t+WV)
                                pstride = (R if x_merge else 1) * PPC * WX
                                xap = AP(
                                    xb.tensor, xb.offset,
                                    [[pstride, 128], [2, cnt], [WX, PPC], [1, WV]],
                                )
                                nc.vector.tensor_mul(
                                    prod[:], wg[:, t0 : t0 + cnt], xap
                                )
                                for m in range(cnt):
                                    for s in range(NSEG):
                                        nc.tensor.matmul(
                                            psums[s][:],
                                            ident[:],
                                            prod[:, m, 2 * s : 2 * s + 2, off : off + W],
                                            start=(i == 0 and gi == 0 and m == 0),
                                            stop=(
                                                i == R - 1 and gi == 1 and m == cnt - 1
                                            ),
                                        )
                    wgs = {}  # i -> row-group weight tile (row_group mode)
                    for k in range(R * R if not packed else 0):
                        i, j = divmod(k, R)
                        need_row(i)
                        if row_group:
                            # one 2.8 MB DMA per tap-row: the 7 taps of row i
                            # are contiguous in the host weight layout
                            if i not in wgs:
                                wg = pool.tile(
                                    [128, R, PPC, WV], dt,
                                    name=f"w_{rep}_{blk}_{i}", tag="wg", bufs=bufs_wg,
                                )
                                weng.dma_start(
                                    out=wg[:], in_=wt[r0 : r0 + 128, R * i : R * i + R]
                                )
                                wgs[i] = wg
                            wk_ap = wgs[i][:, j]
                        else:
                            wk = pool.tile(
                                [128, PPC, WV], dt,
                                name=f"w_{rep}_{blk}_{k}", tag="wk", bufs=6,
                            )
                            weng.dma_start(out=wk[:], in_=wt[r0 : r0 + 128, k])
                            wk_ap = wk[:]
                        if dma_only:
                            continue
                        prod = pool.tile(
                            [128, PPC, WV], dt,
                            name=f"p_{rep}_{blk}_{k}", tag="prod", bufs=4,
                        )
                        c0 = j - (j % 2)  # even latent col offset
                        nc.vector.tensor_mul(
                            prod[:], wk_ap, rs[i][:, :, c0 : c0 + WV]
                        )
                        off = j % 2  # product cols [off, off+W) hold out cols [0, W)
                        for s in range(NSEG):
                            nc.tensor.matmul(
                                psums[s][:],
                                ident[:],
                                prod[:, 2 * s : 2 * s + 2, off : off + W],
                                start=(k == 0),
                                stop=(k == R * R - 1),
                            )
                    outsb = pool.tile(
                        [128, PPC, W], dt,
                        name=f"o_{rep}_{blk}", tag="outsb", bufs=2,
                    )
                    if dma_only:
                        nc.gpsimd.memset(outsb[:], 0.0)
                    else:
                        for s in range(NSEG):
                            nc.scalar.copy(
                                outsb[:, 2 * s : 2 * s + 2, :],
                                psums[s][:].rearrange("p (a b) -> p a b", a=2),
                            )
                    nc.scalar.dma_start(out=out_r[r0 : r0 + 128], in_=outsb[:])

            if loop_reps is not None:
                with tc.For_i(0, loop_reps, 1, staggered_reset=staggered):
                    rep_body(0)
            else:
                for rep in range(reps):
                    rep_body(rep)
    if legalize:
        _split_multi_waits(nc)
    return nc


def _prep_inputs(latent, weights, wdtype: str = WDTYPE, packed: bool = PACKED):
    lat = np.asarray(latent, dtype=np.float32).reshape(PLANES, H, W)
    wts = np.asarray(weights, dtype=np.float32).reshape(PLANES, R * R, H, W)
    if wdtype == "u8":
        # w ~= q/255 with q uint8; the 1/255 is folded into the latent below
        lat = lat * np.float32(1.0 / 255.0)
    # Edge-pad latent: rows 3/3, cols 3 left / 5 right (extra 2 right cols
    # keep the 258-wide device reads in-bounds; values there only feed
    # zero-weight product columns).
    lpad = np.pad(lat, ((0, 0), (PAD, PAD), (PAD, 5)), mode="edge").astype(np.float16)
    odd = (np.arange(R * R) % R) % 2 == 1
    in_maps = []
    for c in range(NCORES):
        wc = wts[c * PPC : (c + 1) * PPC]  # [6, 49, 256, 256]
        wt_t = wc.transpose(2, 1, 0, 3)  # [row, k, plane, col]
        if wdtype == "u8":
            wt_t = np.clip(np.round(wt_t * 255.0), 0, 255)
            v = np.zeros((H, R * R, PPC, WV), dtype=np.uint8)
        else:
            v = np.zeros((H, R * R, PPC, WV), dtype=np.float16)
        v[:, ~odd, :, 0:W] = wt_t[:, ~odd]
        v[:, odd, :, 1 : W + 1] = wt_t[:, odd]  # odd col taps pre-shifted
        if packed:
            perm = [R * i + j for i in range(R) for j in TAP_ORDER]
            v = v[:, perm]
        in_maps.append(
            {
                "wt": np.ascontiguousarray(v),
                "lp": np.ascontiguousarray(
                    lpad[c * PPC : (c + 1) * PPC].transpose(1, 0, 2)
                ),
            }
        )
    return in_maps


def _get_runner():
    """Build the Bass program and ONE sharded jit executable, cached for the
    process.  Repeated kernel() calls reuse the same loaded executable —
    creating a fresh jit per call (as run_bass_kernel_spmd does) loads a new
    executable each time and can wedge the device on the second call."""
    if "runner" in _cache:
        return _cache["runner"]

    import jax
    import concourse.mybir as mybir
    from concourse import bass2jax
    from jax.experimental.shard_map import shard_map
    from jax.sharding import Mesh, NamedSharding, PartitionSpec

    bass2jax.install_neuronx_cc_hook()
    nc = build_nc(reps=1)

    partition_name = nc.partition_id_tensor.name if nc.partition_id_tensor else None
    in_names, out_names, out_avals, zero_outs = [], [], [], []
    for alloc in nc.m.functions[0].allocations:
        if not isinstance(alloc, mybir.MemoryLocationSet):
            continue
        name = alloc.memorylocations[0].name
        if alloc.kind == "ExternalInput":
            if name != partition_name:
                in_names.append(name)
        elif alloc.kind == "ExternalOutput":
            out_names.append(name)
            shape = tuple(alloc.tensor_shape)
            dtype = mybir.dt.np(alloc.dtype)
            out_avals.append(jax.core.ShapedArray(shape, dtype))
            zero_outs.append(np.zeros(shape, dtype))
    n_params = len(in_names)
    all_in_names = list(in_names) + out_names
    if partition_name is not None:
        all_in_names.append(partition_name)

    def _body(*args):
        operands = list(args)
        if partition_name is not None:
            operands.append(bass2jax.partition_id_tensor())
        return tuple(
            bass2jax._bass_exec_p.bind(
                *operands,
                out_avals=tuple(out_avals),
                in_names=tuple(all_in_names),
                out_names=tuple(out_names),
                lowering_input_output_aliases=(),
                sim_require_finite=True,
                sim_require_nnan=True,
                nc=nc,
            )
        )

    devices = jax.devices()[:NCORES]
    mesh = Mesh(np.asarray(devices), ("core",))
    in_specs = (PartitionSpec("core"),) * (n_params + len(out_names))
    out_specs = (PartitionSpec("core"),) * len(out_names)
    sharded = jax.jit(
        shard_map(
            _body, mesh=mesh, in_specs=in_specs, out_specs=out_specs, check_rep=False
        ),
        keep_unused=True,
    )
    sh = NamedSharding(mesh, PartitionSpec("core"))
    zeros_dev = [
        jax.device_put(np.zeros((NCORES * z.shape[0], *z.shape[1:]), z.dtype), sh)
        for z in zero_outs
    ]

    def run(in_maps):
        ins_dev = [
            jax.device_put(
                np.concatenate([in_maps[c][n] for c in range(NCORES)], axis=0), sh
            )
            for n in in_names
        ]
        outs = sharded(*ins_dev, *zeros_dev)
        jax.block_until_ready(outs)
        # one output tensor: per-core [PPC, H, W] concatenated on axis 0
        return np.asarray(outs[0])

    _cache["runner"] = run
    return run


def kernel(latent, weights, window_size):
    r = int(window_size)
    assert r == R, f"kernel hardcoded for window_size={R}, got {r}"

    run = _get_runner()
    in_maps = _prep_inputs(latent, weights)
    full = run(in_maps)  # [NCORES*H, PPC, W]
    full = full.reshape(NCORES, H, PPC, W).transpose(0, 2, 1, 3)
    return full.reshape(B, C, H, W).astype(np.float32)
